# revision 58
# baseline (speedup 1.0000x reference)
"""Trainium2 Bass kernel for nn_MDRMWithCPRecon (v2).

Sharding: pure data parallel over batch B=8 -> one batch element per
NeuronCore. Each core computes the full per-batch pipeline.

Differences vs v1 (294us):
  - bf16 conv (weights + padded image); image loaded via SWDGE cast-DMA
    (f32 DRAM -> bf16 SBUF) in row-chunks so the conv starts at ~7us.
  - Fm / A / t2 kept in SBUF in bf16 (no DRAM scratch round trip);
    A = alpha*frm - t2 and t2 = (1-alpha)*oth precomputed in the conv
    shadow.
  - Pooled stats split across engines: per-tile row sums via activation
    accum, q-stats on DVE, p-stats on GpSimd; global max derived from
    q_max (no per-tile full reduce).
  - adapter+U_gen collapsed host-side: u = wv*avg + wm*mx + c0 (rank-2
    affine of the pooled rows); softmax without max-subtraction (logits
    are tiny by construction).
  - spectral scale, lambda and recon bias folded into a 5-row matmul:
    rec = MT8^T @ Gp with G8[4]=1, MT8[4]=br*spectral, lam8=[lam;1;0;0;0];
    the spatial sigmoid is folded into Gp and +Fm is added by an identity
    matmul, so the whole cp_recon tile is produced in PSUM by the PE.
  - final elementwise stage in bf16, split DVE / GpSimd / Scalar.
"""

import numpy as np
import ml_dtypes

import concourse.bacc as bacc
import concourse.bass as bass
import concourse.tile as tile
from concourse import mybir, bass_utils

F32 = mybir.dt.float32
BF16 = mybir.dt.bfloat16
AF = mybir.ActivationFunctionType
ALU = mybir.AluOpType
AX = mybir.AxisListType

B, C, H, W, K = 8, 256, 64, 64, 4
HW = H * W
NCORES = 8
BF = ml_dtypes.bfloat16


def build_program(alpha, ws, bs):
    from concourse.masks import make_identity

    nc = bacc.Bacc("TRN2", target_bir_lowering=False, debug=False,
                   num_devices=NCORES)

    frm = nc.dram_tensor("frm", [C, H, W], F32, kind="ExternalInput")
    oth = nc.dram_tensor("oth", [C, H, W], F32, kind="ExternalInput")
    w3t_d = nc.dram_tensor("w3t", [128, 4, 9, 256], BF16, kind="ExternalInput")
    b3_d = nc.dram_tensor("b3", [128, 2], F32, kind="ExternalInput")
    uvw_d = nc.dram_tensor("uvw", [1, 3, 2, 4], BF16, kind="ExternalInput")
    c0_d = nc.dram_tensor("c0", [4, 3], F32, kind="ExternalInput")
    wrt_d = nc.dram_tensor("wrt", [128, 2, 2, 128], BF16, kind="ExternalInput")
    br_d = nc.dram_tensor("br", [128, 2], F32, kind="ExternalInput")
    wsc_d = nc.dram_tensor("wsc", [128, 4, 2, 128], F32, kind="ExternalInput")
    bsc_d = nc.dram_tensor("bsc", [128, 2], F32, kind="ExternalInput")
    lam_d = nc.dram_tensor("lam", [8, 1], F32, kind="ExternalInput")
    fused_o = nc.dram_tensor("fused", [C, H, W], F32, kind="ExternalOutput")
    cpr_o = nc.dram_tensor("cpr", [C, H, W], F32, kind="ExternalOutput")

    with tile.TileContext(nc) as tc:
        _build_tile(tc, nc, make_identity, locals(), alpha, ws, bs)
    nc.compile()
    return nc


def _build_tile(tc, nc, make_identity, T, alpha, ws, bs):
    frm, oth = T["frm"], T["oth"]
    w3t_d, b3_d, uvw_d, c0_d = T["w3t_d"], T["b3_d"], T["uvw_d"], T["c0_d"]
    wrt_d, br_d, wsc_d, bsc_d = T["wrt_d"], T["br_d"], T["wsc_d"], T["bsc_d"]
    lam_d = T["lam_d"]
    fused_o, cpr_o = T["fused_o"], T["cpr_o"]

    import contextlib
    ctx = contextlib.ExitStack()
    consts = ctx.enter_context(tc.tile_pool(name="consts", bufs=1))
    ew = ctx.enter_context(tc.tile_pool(name="ew", bufs=2))
    outr = ctx.enter_context(tc.tile_pool(name="outr", bufs=2))
    ps_conv = ctx.enter_context(tc.tile_pool(name="ps_conv", bufs=2, space="PSUM"))
    ps_spat = ctx.enter_context(tc.tile_pool(name="ps_spat", bufs=2, space="PSUM"))
    ps_rec = ctx.enter_context(tc.tile_pool(name="ps_rec", bufs=2, space="PSUM"))
    ps_sm = ctx.enter_context(tc.tile_pool(name="ps_sm", bufs=2, space="PSUM"))

    stage = ctx.enter_context(tc.tile_pool(name="stage", bufs=4))

    # ---- padded image, bf16 [128, 4(kt), 66, 68]; interior at rows 1..64,
    # cols 2..65; zero border at rows 0/65, cols 1/66 (0 and 67 unused) ----
    xr = consts.tile([128, 4, 66, 68], BF16)
    # border memsets on DVE: ~340ns each vs 1.2-1.6us on gpsimd, and they
    # gate the first conv matmul
    nc.vector.memset(xr[:, :, :, 1:2], 0.0)
    nc.vector.memset(xr[:, :, :, 66:67], 0.0)
    nc.vector.memset(xr[:, :, 0:1, 1:67], 0.0)
    nc.vector.memset(xr[:, :, 65:66, 1:67], 0.0)

    # inputs: HWDGE f32 DMA into a staging ring + scalar-engine convert to
    # the padded bf16 image. Row chunks sized so conv tile pt only needs
    # the chunks covering its rows; q0 is small for an early conv start.
    chunks = [(0, 9), (8, 34), (32, 58), (56, 64)]
    frm_v = frm.rearrange("(k p) h w -> p k h w", p=128)
    oth_v = oth.rearrange("(k p) h w -> p k h w", p=128)

    def load_chunk(q):
        slo, shi = chunks[q]
        for kt in range(4):
            src_v = (frm_v, oth_v)[kt // 2]
            stg = stage.tile([128, 26, 64], F32, tag="stg")
            nc.sync.dma_start(stg[:, 0:shi - slo, :],
                              src_v[:, kt % 2, slo:shi, :])
            # q0 gates the first conv matmul: split its converts across
            # two engines so the chains run in parallel
            if q == 0 and kt >= 2:
                nc.vector.tensor_copy(xr[:, kt, slo + 1:shi + 1, 2:66],
                                      stg[:, 0:shi - slo, :])
            else:
                nc.scalar.copy(xr[:, kt, slo + 1:shi + 1, 2:66],
                               stg[:, 0:shi - slo, :])

    # Each HWDGE dma_start costs ~0.7us of issue time on the issuing
    # queue, and concurrent transfers share HBM bandwidth round-robin.
    # Issue the critical q0 image chunks on Sync and the per-kt weight
    # chunks on Scalar (both are HWDGE) so issue streams run in parallel
    # and each conv matmul's own (kt) dependencies land first.
    w3t_sb = consts.tile([128, 4, 9, 256], BF16)
    for kt in range(4):
        nc.scalar.dma_start(w3t_sb[:, kt], w3t_d[:, kt])
    load_chunk(0)
    b3_sb = consts.tile([128, 2], F32)
    nc.sync.dma_start(b3_sb[:], b3_d[:])
    load_chunk(1)

    # ---- small consts ----
    uvw_sb = consts.tile([1, 3, 2, 4], BF16)
    nc.sync.dma_start(uvw_sb[:], uvw_d[:])
    c0_sb = consts.tile([4, 3], F32)
    nc.sync.dma_start(c0_sb[:], c0_d[:])
    wrt_sb = consts.tile([128, 2, 2, 128], BF16)
    nc.sync.dma_start(wrt_sb[:], wrt_d[:])
    br_sb = consts.tile([128, 2], F32)
    nc.sync.dma_start(br_sb[:], br_d[:])
    wsc_sb = consts.tile([128, 4, 2, 128], F32)
    nc.sync.dma_start(wsc_sb[:], wsc_d[:])
    bsc_sb = consts.tile([128, 2], F32)
    nc.sync.dma_start(bsc_sb[:], bsc_d[:])
    lam_sb = consts.tile([8, 1], F32)
    nc.sync.dma_start(lam_sb[:], lam_d[:])

    ident = consts.tile([128, 128], F32)
    make_identity(nc, ident[:])
    identb = consts.tile([128, 128], BF16)
    nc.vector.tensor_copy(identb[:], ident[:])
    ones128 = consts.tile([128, 1], F32)
    nc.gpsimd.memset(ones128[:], 1.0)
    ones4b = consts.tile([4, 128], BF16)
    nc.gpsimd.memset(ones4b[:], 1.0)
    # G8 rows 0-3: U3 (x) U2 outer product (written in the middle phase);
    # row 4 stays 1.0 from this early memset (carries the recon bias via
    # MT8 row 4); rows 5-7 are junk killed by lam8 = 0.
    G8 = consts.tile([8, 4096], BF16)
    nc.vector.memset(G8[:], 1.0)

    # ---- persistent per-batch state ----
    fm_sb = consts.tile([128, 2, 4096], BF16)   # Fm, (ct, h, w) layout
    A_sb = consts.tile([128, 2, 4096], BF16)    # alpha*frm - t2
    t2_sb = consts.tile([128, 2, 4096], BF16)   # (1-alpha)*oth
    sums1 = consts.tile([128, 2, 8], F32)       # per-tile Fm sums (accum)
    q_sum = consts.tile([128, 2, 64], F32)      # sum over w -> [c, h]
    q_max = consts.tile([128, 2, 64], F32)
    pp_sum = consts.tile([128, 2, 64, 8], F32)  # per-pt sum over h -> [c, w]
    pp_max = consts.tile([128, 2, 64, 8], F32)

    def emit_a_t2():
        # A / t2 precompute; emitted mid-conv so the strict-FIFO scalar/
        # vector queues are not blocked waiting for the full image load.
        for ct in range(2):
            frm_v = xr[:, ct, 1:65, 2:66]
            oth_v = xr[:, 2 + ct, 1:65, 2:66]
            t2v = t2_sb[:, ct].rearrange("p (h w) -> p h w", h=64)
            nc.scalar.activation(t2v, oth_v, AF.Copy,
                                 scale=float(1.0 - alpha))
            nc.vector.scalar_tensor_tensor(
                A_sb[:, ct].rearrange("p (h w) -> p h w", h=64),
                frm_v, float(alpha), t2v, op0=ALU.mult, op1=ALU.subtract)

    # partial p-stat combines over tiles 0..6, run hidden under the last
    # conv block; only a cheap merge with the pt7 slice remains at the end
    p_sum_a = consts.tile([128, 2, 64], F32)
    p_max_a = consts.tile([128, 2, 64], F32)

    # ---- conv3x3 (bf16) + leaky relu + streaming stats ----
    for pt in range(8):
        if pt == 1:
            load_chunk(2)
        if pt == 3:
            load_chunk(3)
        if pt == 5:
            emit_a_t2()
        if pt == 7:
            nc.vector.tensor_reduce(p_sum_a[:], pp_sum[:, :, :, 0:7],
                                    axis=AX.X, op=ALU.add)
            nc.vector.tensor_reduce(p_max_a[:], pp_max[:, :, :, 0:7],
                                    axis=AX.X, op=ALU.max)
        for ct in range(2):
            ps = ps_conv.tile([128, 512], F32, tag="conv")
            idx = 0
            for kt in range(4):
                for t in range(9):
                    dy, dx = t // 3, t % 3
                    nc.tensor.matmul(
                        ps[:],
                        w3t_sb[:, kt, t, ct * 128:(ct + 1) * 128],
                        xr[:, kt, pt * 8 + dy: pt * 8 + dy + 8,
                           dx + 1: dx + 65],
                        start=(idx == 0), stop=(idx == 35))
                    idx += 1
            fm_t = fm_sb[:, ct, pt * 512:(pt + 1) * 512]
            nc.scalar.activation(fm_t, ps[:], AF.Lrelu,
                                 bias=b3_sb[:, ct:ct + 1], alpha=0.01,
                                 accum_out=sums1[:, ct, pt:pt + 1])
            blk = fm_t.rearrange("p (h w) -> p h w", h=8)
            blk_t = fm_t.rearrange("p (h w) -> p w h", h=8)
            # pt7: p-stats first — they gate the post-conv merge chain
            if pt == 7:
                nc.vector.tensor_reduce(pp_sum[:, ct, :, pt], blk_t,
                                        axis=AX.X, op=ALU.add)
                nc.vector.tensor_reduce(pp_max[:, ct, :, pt], blk_t,
                                        axis=AX.X, op=ALU.max)
            nc.vector.tensor_reduce(q_sum[:, ct, pt * 8:(pt + 1) * 8], blk,
                                    axis=AX.X, op=ALU.add)
            nc.vector.tensor_reduce(q_max[:, ct, pt * 8:(pt + 1) * 8], blk,
                                    axis=AX.X, op=ALU.max)
            if pt != 7:
                nc.vector.tensor_reduce(pp_sum[:, ct, :, pt], blk_t,
                                        axis=AX.X, op=ALU.add)
                nc.vector.tensor_reduce(pp_max[:, ct, :, pt], blk_t,
                                        axis=AX.X, op=ALU.max)

    # ---- preload the EXP activation table while the conv tail drains;
    # reading sums1 pins this after the last conv Lrelu so the table
    # sequence stays LRELU -> EXP -> SIGMOID with no thrash ----
    dummy_e = ew.tile([1, 1], F32, tag="dummy")
    nc.scalar.activation(dummy_e[:], sums1[0:1, 0, 0:1], AF.Exp)

    # ---- combine partials (p-stats: cheap merge of the precombined
    # 0..6 partials with the pt7 slice) ----
    p_sum = consts.tile([128, 2, 64], F32)
    nc.vector.tensor_tensor(p_sum[:], p_sum_a[:], pp_sum[:, :, :, 7],
                            op=ALU.add)
    p_max = consts.tile([128, 2, 64], F32)
    nc.vector.tensor_tensor(p_max[:], p_max_a[:], pp_max[:, :, :, 7],
                            op=ALU.max)
    sum1 = consts.tile([128, 2], F32)
    nc.vector.tensor_reduce(sum1[:], sums1[:], axis=AX.X, op=ALU.add)
    max1 = consts.tile([128, 2], F32)
    nc.vector.tensor_reduce(max1[:], q_max[:], axis=AX.X, op=ALU.max)

    def colstats(S_sum, S_max, uid):
        avg_row = consts.tile([1, 64], BF16, tag=f"avgr{uid}")
        max_row = consts.tile([1, 64], BF16, tag=f"maxr{uid}")
        ssum = ps_sm.tile([1, 64], F32, tag="sm")
        nc.tensor.matmul(ssum[:], ones128[:], S_sum[:, 0, :], start=True,
                         stop=False)
        nc.tensor.matmul(ssum[:], ones128[:], S_sum[:, 1, :], start=False,
                         stop=True)
        nc.scalar.mul(avg_row[:], ssum[:], 1.0 / (C * H))
        mx = ew.tile([128, 64], F32, tag="mx")
        nc.vector.tensor_tensor(mx[:], S_max[:, 0, :], S_max[:, 1, :],
                                op=ALU.max)
        mxt = ps_sm.tile([64, 128], F32, tag="sm")
        nc.tensor.transpose(mxt[:], mx[:], ident[:])
        mxr = ew.tile([64, 1], F32, tag="mxr")
        nc.vector.tensor_reduce(mxr[:], mxt[:], axis=AX.X, op=ALU.max)
        mxp = ps_sm.tile([1, 64], F32, tag="sm")
        nc.tensor.transpose(mxp[:], mxr[:], ident[0:64, 0:64])
        nc.scalar.copy(max_row[:], mxp[:])
        return avg_row, max_row

    avg_row2, max_row2 = colstats(p_sum, p_max, "m2")
    avg_row3, max_row3 = colstats(q_sum, q_max, "m3")

    # ---- U factors: u = wv*avg + wm*mx + c0; softmax over k=4 (no
    # max-subtraction: |u| << 1 by construction) ----
    U1n = consts.tile([128, 2, 4], BF16)

    def make_U(m, avg_row, max_row, N, need_un):
        u_ps = ps_sm.tile([4, N], F32, tag="sm")
        nc.tensor.matmul(u_ps[:], uvw_sb[0:1, m, 0, :], avg_row[:],
                         start=True, stop=False)
        nc.tensor.matmul(u_ps[:], uvw_sb[0:1, m, 1, :], max_row[:],
                         start=False, stop=True)
        e = ew.tile([4, N], F32, tag=f"e{m}")
        nc.scalar.activation(e[:], u_ps[:], AF.Exp, bias=c0_sb[:, m:m + 1])
        UT = consts.tile([4, N], BF16, tag=f"UT{m}")
        nch = max(N // 128, 1)
        cw = min(N, 128)
        for ch in range(nch):
            et_ps = ps_sm.tile([cw, 4], F32, tag="sm")
            nc.tensor.transpose(et_ps[:], e[0:4, ch * cw:(ch + 1) * cw],
                                ident[0:4, 0:4])
            ssum = ew.tile([cw, 1], F32, tag="ssum")
            nc.vector.tensor_reduce(ssum[:], et_ps[:], axis=AX.X, op=ALU.add)
            rec = ew.tile([cw, 1], F32, tag="rec")
            nc.vector.reciprocal(rec[:], ssum[:])
            un = ew.tile([cw, 4], F32, tag=f"un{m}")
            nc.vector.tensor_scalar(un[:], et_ps[:], rec[:], None,
                                    op0=ALU.mult)
            if need_un:
                nc.vector.tensor_copy(U1n[:, ch, :], un[:])
            ut_ps = ps_sm.tile([4, cw], F32, tag="sm")
            nc.tensor.transpose(ut_ps[:], un[:], ident[0:cw, 0:cw])
            nc.vector.tensor_copy(UT[:, ch * cw:(ch + 1) * cw], ut_ps[:])
        return UT

    # U2/U3 first: they gate the G8 outer-product and the spatial map.
    U2T = make_U(1, avg_row2, max_row2, 64, False)
    U3T = make_U(2, avg_row3, max_row3, 64, False)

    # ---- G8[r, h, w] = U3T[r, h] * U2T[r, w] (rows 0-3), bf16. All on
    # DVE: gpsimd tensor ops contend for SBUF ports and slow concurrent
    # DVE ops ~2x, and a 4-partition gpsimd op uses one Q7 core anyway ----
    nc.vector.tensor_tensor(
        G8[0:4, :].rearrange("p (h w) -> p h w", h=64),
        U3T[:, 0:64][:, :, None].broadcast_to([4, 64, 64]),
        U2T[:, 0:64][:, None, :].broadcast_to([4, 64, 64]),
        op=ALU.mult)

    # ---- mode-1 pooled rows + U1 ----
    avg_row1 = consts.tile([1, 256], BF16)
    max_row1 = consts.tile([1, 256], BF16)
    for ct in range(2):
        tp = ps_sm.tile([1, 128], F32, tag="sm")
        nc.tensor.transpose(tp[:], sum1[:, ct:ct + 1], ident[:])
        nc.scalar.mul(avg_row1[0:1, ct * 128:(ct + 1) * 128], tp[:], 1.0 / HW)
        tp2 = ps_sm.tile([1, 128], F32, tag="sm")
        nc.tensor.transpose(tp2[:], max1[:, ct:ct + 1], ident[:])
        nc.scalar.copy(max_row1[0:1, ct * 128:(ct + 1) * 128], tp2[:])
    U1T = make_U(0, avg_row1, max_row1, 256, True)

    # ---- spectral attention -> [128, 2] f32 ----
    gag = consts.tile([128, 4], F32)  # [ga_ct0, ga_ct1, gm_ct0, gm_ct1]
    for ct in range(2):
        f_ps = ps_sm.tile([128, 128], F32, tag="sm")
        nc.tensor.matmul(f_ps[:, 0:64], U1T[:, ct * 128:(ct + 1) * 128],
                         U2T[:], start=True, stop=True)
        nc.tensor.matmul(f_ps[:, 64:128], U1T[:, ct * 128:(ct + 1) * 128],
                         U3T[:], start=True, stop=True)
        nc.vector.tensor_reduce(gag[:, ct:ct + 1], f_ps[:], axis=AX.X,
                                op=ALU.add)
        nc.vector.tensor_reduce(gag[:, 2 + ct:3 + ct], f_ps[:], axis=AX.X,
                                op=ALU.max)
    spectral = consts.tile([128, 2], F32)
    for mh in range(2):
        sp_ps = ps_sm.tile([128, 1], F32, tag="sm")
        for kk in range(4):
            nc.tensor.matmul(sp_ps[:], wsc_sb[:, kk, mh, :],
                             gag[:, kk:kk + 1], start=(kk == 0),
                             stop=(kk == 3))
        stmp = ew.tile([128, 1], F32, tag="stmp")
        nc.scalar.activation(stmp[:], sp_ps[:], AF.Sigmoid,
                             bias=bsc_sb[:, mh:mh + 1])
        nc.scalar.activation(spectral[:, mh:mh + 1], stmp[:], AF.Sigmoid)

    # ---- MT8 [8, 256] bf16: rows 0-3 = (spc * Wr @ U1 diag(lam))^T,
    # row 4 = spc * br (bias, paired with G8 row 4 == 1), rows 5-7
    # zeroed by lam8 = [lam; 1; 0; 0; 0] ----
    MT8 = consts.tile([8, 256], BF16)
    msc8 = consts.tile([128, 8], F32)
    for mh in range(2):
        m_ps = ps_sm.tile([128, 4], F32, tag="sm")
        for kk in range(2):
            nc.tensor.matmul(m_ps[:], wrt_sb[:, kk, mh, :], U1n[:, kk, :],
                             start=(kk == 0), stop=(kk == 1))
        nc.vector.tensor_scalar(msc8[:, 0:4], m_ps[:],
                                spectral[:, mh:mh + 1], None, op0=ALU.mult)
        nc.vector.tensor_scalar(msc8[:, 4:5], br_sb[:, mh:mh + 1],
                                spectral[:, mh:mh + 1], None, op0=ALU.mult)
        nc.vector.memset(msc8[:, 5:8], 0.0)
        mt_ps = ps_sm.tile([8, 128], F32, tag="sm")
        nc.tensor.transpose(mt_ps[:], msc8[:], ident[:])
        nc.vector.tensor_scalar(MT8[:, mh * 128:(mh + 1) * 128], mt_ps[:],
                                lam_sb[:], None, op0=ALU.mult)

    # ---- spatial sigmoid map, bf16 [128, 4096] ----
    sig_sp = consts.tile([128, 4096], BF16)
    for q in range(8):
        sp_ps = ps_spat.tile([128, 512], F32, tag="spat")
        nc.tensor.matmul(sp_ps[:], ones4b[:], G8[0:4, q * 512:(q + 1) * 512],
                         start=True, stop=True)
        nc.scalar.activation(sig_sp[:, q * 512:(q + 1) * 512], sp_ps[:],
                             AF.Sigmoid, scale=float(ws), bias=float(bs))

    # ---- final elementwise stage ----
    # cp = (spc*(Wr@cp_core) + spc*br) * sig + Fm is computed entirely in
    # PSUM by the tensor engine: sig is folded into G per pt-block
    # (Gp = G * sig rows 0:4), the bias via a rank-1 matmul against the
    # partition-0 sig row, and +Fm via an identity matmul. The scalar
    # engine evicts the finished tile. DVE only does the fused-side ops.
    fused_v = fused_o.rearrange("(k p) h w -> p k h w", p=128)
    cpr_v = cpr_o.rearrange("(k p) h w -> p k h w", p=128)
    for pt in range(8):
        sl = slice(pt * 512, (pt + 1) * 512)
        sig_v = sig_sp[:, sl]
        Gp = ew.tile([8, 512], BF16, tag="Gp")
        nc.vector.tensor_tensor(Gp[:], G8[:, sl], sig_sp[0:8, sl],
                                op=ALU.mult)
        fu = outr.tile([128, 2, 512], F32, tag="fu")
        cp = outr.tile([128, 2, 512], F32, tag="cp")
        for ct in range(2):
            spc = spectral[:, ct:ct + 1]
            rc_ps = ps_rec.tile([128, 512], F32, tag="rec")
            nc.tensor.matmul(rc_ps[:], MT8[:, ct * 128:(ct + 1) * 128],
                             Gp[:], start=True, stop=False)
            nc.tensor.matmul(rc_ps[:], identb[:], fm_sb[:, ct, sl],
                             start=False, stop=True)
            nc.scalar.copy(cp[:, ct, :], rc_ps[:])
            A2 = ew.tile([128, 512], BF16, tag="A2")
            nc.vector.scalar_tensor_tensor(A2[:], A_sb[:, ct, sl], spc,
                                           sig_v, op0=ALU.mult, op1=ALU.mult)
            nc.vector.tensor_tensor(fu[:, ct, :], A2[:], t2_sb[:, ct, sl],
                                    op=ALU.add)
        nc.sync.dma_start(
            fused_v[:, :, pt * 8:(pt + 1) * 8, :],
            fu[:].rearrange("p k (h w) -> p k h w", h=8))
        nc.sync.dma_start(
            cpr_v[:, :, pt * 8:(pt + 1) * 8, :],
            cp[:].rearrange("p k (h w) -> p k h w", h=8))
    ctx.close()


def _prep_weights(W3, b3, Wa1, ba1, Wa2, ba2, Wa3, ba3, Wu, bu, Wr, br,
                  Wsa, bsa, Wsm, bsm):
    f = np.float32
    w3t = np.ascontiguousarray(
        W3.reshape(C, 4, 128, 9).transpose(2, 1, 3, 0)).astype(BF)
    b3h = np.ascontiguousarray(b3.reshape(2, 128).T).astype(f)
    # u = wv*avg + wm*mx + c0 with wv_m = Wu @ Wa_m[:,0], etc.
    was = np.stack([Wa1, Wa2, Wa3], axis=0).astype(np.float64)  # [3, C, 2]
    bas = np.stack([ba1, ba2, ba3], axis=0).astype(np.float64)  # [3, C]
    Wu64 = Wu.astype(np.float64)
    uvw = np.einsum("kc,mcs->msk", Wu64, was)                   # [3, 2, 4]
    c0 = (np.einsum("kc,mc->mk", Wu64, bas) + bu[None, :]).T    # [4, 3]
    uvwh = np.ascontiguousarray(uvw[None]).astype(BF)
    c0h = np.ascontiguousarray(c0).astype(f)
    wrt = np.ascontiguousarray(
        Wr.reshape(2, 128, 2, 128).transpose(3, 2, 0, 1)).astype(BF)
    brh = np.ascontiguousarray(br.reshape(2, 128).T).astype(f)
    wsa_r = (Wsa / 128.0).reshape(2, 128, 2, 128).transpose(3, 2, 0, 1)
    wsm_r = Wsm.reshape(2, 128, 2, 128).transpose(3, 2, 0, 1)
    wsc = np.ascontiguousarray(
        np.concatenate([wsa_r, wsm_r], axis=1)).astype(f)
    bsc = np.ascontiguousarray((bsa + bsm).reshape(2, 128).T).astype(f)
    return dict(w3t=w3t, b3=b3h, uvw=uvwh, c0=c0h, wrt=wrt, br=brh,
                wsc=wsc, bsc=bsc)


_CACHE = {}


def kernel(frm_feat, other_feat, W3, b3, Wa1, ba1, Wa2, ba2, Wa3, ba3,
           Wu, bu, Wr, br, ws, bs, Wsa, bsa, Wsm, bsm, alpha, lam,
           _trace=False, _tmpdir=None):
    frm_feat = np.asarray(frm_feat, np.float32)
    other_feat = np.asarray(other_feat, np.float32)
    key = (float(alpha), float(ws), float(bs))
    if key not in _CACHE:
        _CACHE[key] = build_program(float(alpha), float(ws), float(bs))
    nc = _CACHE[key]

    wd = _prep_weights(np.asarray(W3), np.asarray(b3), np.asarray(Wa1),
                       np.asarray(ba1), np.asarray(Wa2), np.asarray(ba2),
                       np.asarray(Wa3), np.asarray(ba3), np.asarray(Wu),
                       np.asarray(bu), np.asarray(Wr), np.asarray(br),
                       np.asarray(Wsa), np.asarray(bsa), np.asarray(Wsm),
                       np.asarray(bsm))
    lam8 = np.zeros((8, 1), np.float32)
    lam8[0:4, 0] = np.asarray(lam, np.float32).reshape(4)
    lam8[4, 0] = 1.0
    wd["lam"] = lam8

    in_maps = []
    for b_i in range(NCORES):
        m = dict(wd)
        m["frm"] = np.ascontiguousarray(frm_feat[b_i])
        m["oth"] = np.ascontiguousarray(other_feat[b_i])
        in_maps.append(m)

    res = bass_utils.run_bass_kernel_spmd(
        nc, in_maps, core_ids=list(range(NCORES)), trace=_trace,
        tmpdir=_tmpdir)
    fused = np.stack([res.results[i]["fused"] for i in range(NCORES)])
    cpr = np.stack([res.results[i]["cpr"] for i in range(NCORES)])
    kernel._last_exec_time_ns = res.exec_time_ns
    kernel._last_results = res
    return fused, cpr
